# revision 22
# baseline (speedup 1.0000x reference)
"""Trainium2 Bass kernel for the MemoryEfficientMambaBlock problem.

Data-parallel over 8 NeuronCores: x sharded over tokens, small weights
replicated. Per core, per 256-token tile:
  LayerNorm (bn_stats token-major; rstd via DVE Newton iteration so the
  ACT engine never swaps activation tables) -> transpose to feature-major
  via regular bf16 matmuls against the identity (gamma/beta fused into the
  PSUM copyback, output quantized bf16) -> bf16 matmul x@W_projT with
  SiLU+b_proj fused into the ACT copyback -> bf16 matmul @W_stateT with
  SiLU+(b_state+initial_state) fused -> K=9 bf16 matmul (ones row carries
  b_out) producing token-major output with the residual add fused into the
  DVE copyback.
"""

import sys

if "/opt/trn_rl_repo" not in sys.path:
    sys.path.insert(0, "/opt/trn_rl_repo")

import ml_dtypes
import numpy as np

import concourse.bass as bass
import concourse.mybir as mybir
import concourse.tile as tile
from concourse.bass_utils import run_bass_kernel_spmd

P = 128
D_MODEL = 1024
D_INNER = 2048
D_STATE = 8
EPS = 1e-5
N_CORES = 8
TOK_TOTAL = 2 * 128 * 196  # 50176
TOK = TOK_TOTAL // N_CORES  # 6272
TILE_T = 512

KD = D_MODEL // P  # 8 contraction chunks for matmul 1
ME = D_INNER // P  # 16 output tiles for matmul 1 / contraction chunks for 2

F32 = mybir.dt.float32
BF16 = mybir.dt.bfloat16


def _split_multi_waits(nc):
    """This container's walrus accepts at most ONE semaphore wait per
    instruction. Hoist all but the last wait of each instruction onto
    fresh single-wait NoOps inserted immediately before it on the same
    engine (the sequencer processes instructions in order, so semantics
    are unchanged)."""
    n_split = 0
    for f in nc.m.functions:
        for blk in f.blocks:
            out = []
            changed = False
            for inst in blk.instructions:
                si = inst.sync_info
                waits = list(si.on_wait) if si is not None else []
                if len(waits) > 1:
                    changed = True
                    for j, w in enumerate(waits[:-1]):
                        nop = mybir.InstNoOp(
                            name=f"{inst.name}-wsplit{j}", ins=[], outs=[]
                        )
                        nop.engine = inst.engine
                        nop.sync_info = mybir.SyncInfo(on_wait=[w], on_update=[])
                        out.append(nop)
                        n_split += 1
                    inst.sync_info = mybir.SyncInfo(
                        on_wait=[waits[-1]], on_update=list(si.on_update)
                    )
                out.append(inst)
            if changed:
                blk.instructions = out
    return n_split


def build_kernel():
    nc = bass.Bass()
    x = nc.dram_tensor("x", [TOK, D_MODEL], F32, kind="ExternalInput")
    wpt = nc.dram_tensor("wpt", [D_MODEL, D_INNER], BF16, kind="ExternalInput")
    wst = nc.dram_tensor("wst", [D_INNER, D_STATE], BF16, kind="ExternalInput")
    wo9 = nc.dram_tensor("wo9", [D_STATE + 1, D_MODEL], BF16, kind="ExternalInput")
    gpk = nc.dram_tensor("gpk", [P, KD], F32, kind="ExternalInput")
    bpk = nc.dram_tensor("bpk", [P, KD], F32, kind="ExternalInput")
    bpm = nc.dram_tensor("bpm", [P, ME], F32, kind="ExternalInput")
    b2 = nc.dram_tensor("b2", [D_STATE, 1], F32, kind="ExternalInput")
    ones = nc.dram_tensor("ones", [1, TILE_T], BF16, kind="ExternalInput")
    ident_d = nc.dram_tensor("ident", [P, P], BF16, kind="ExternalInput")
    y = nc.dram_tensor("y", [TOK, D_MODEL], F32, kind="ExternalOutput")

    # bf16 matmuls run at 1 cycle/row at any N, so tiles need not be
    # uniform: small tiles up front shorten the prologue (first matmul
    # starts after ~one LN chain instead of a whole 512-token tile; they
    # also absorb the PE DVFS ramp), the 128-token remainder gets its
    # own tile instead of a recompute-overlap tile
    tiles = [(0, 128), (128, 384)]
    o = 512
    while o + TILE_T <= TOK:
        tiles.append((o, TILE_T))
        o += TILE_T
    if o < TOK:
        tiles.append((o, TOK - o))

    with tile.TileContext(nc) as tc:
        with (
            tc.tile_pool(name="singles", bufs=1) as singles,
            tc.tile_pool(name="xpool", bufs=3) as xpool,
            tc.tile_pool(name="xnpool", bufs=2) as xnpool,
            tc.tile_pool(name="xtpool", bufs=2) as xtpool,
            tc.tile_pool(name="projp", bufs=2) as projp,
            tc.tile_pool(name="outp", bufs=2) as outp,
            tc.tile_pool(name="statp", bufs=6) as statp,
            tc.tile_pool(name="ps_tr", bufs=2, space="PSUM") as ps_tr,
            tc.tile_pool(name="ps1", bufs=2, space="PSUM") as ps1,
            tc.tile_pool(name="ps2", bufs=1, space="PSUM") as ps2,
            tc.tile_pool(name="ps3", bufs=3, space="PSUM") as ps3,
        ):
            # x tiles 0/1 first: LN work can begin before weights finish.
            # One DMA per 128-token group so bn_stats on group g unblocks
            # as soon as its quarter lands (byte-range dep tracking). A
            # single DMA queue moves only ~115 GB/s, so the startup tiles
            # spread their groups across all three queues.
            def a_dma(off, T, spread=False):
                G = T // P
                x_sb = xpool.tile([P, G, D_MODEL], F32, tag="x")
                xr = x[off : off + T, :].rearrange("(g p) d -> p g d", p=P)
                engs = [nc.sync, nc.scalar] if spread else [nc.sync]
                for g in range(G):
                    engs[g % len(engs)].dma_start(x_sb[:, g], xr[:, g])
                return x_sb

            x_tiles = [a_dma(*tiles[0], spread=True), a_dma(*tiles[1])]

            # weights spread across the scalar/gpsimd DMA queues so they
            # load in parallel with the first x tiles on sync; small
            # tensors first -- the LN->transpose->copyback chain needs
            # ident/gpk/bpk before any wpt chunk is touched
            ident = singles.tile([P, P], BF16)
            nc.gpsimd.dma_start(ident, ident_d[:, :])
            gpk_sb = singles.tile([P, KD], F32)
            nc.scalar.dma_start(gpk_sb, gpk[:, :])
            bpk_sb = singles.tile([P, KD], F32)
            nc.scalar.dma_start(bpk_sb, bpk[:, :])
            bpm_sb = singles.tile([P, ME], F32)
            nc.gpsimd.dma_start(bpm_sb, bpm[:, :])
            b2_sb = singles.tile([D_STATE, 1], F32)
            nc.scalar.dma_start(b2_sb, b2[:, :])
            wst_sb = singles.tile([P, ME, D_STATE], BF16)
            nc.gpsimd.dma_start(wst_sb, wst[:, :].rearrange("(k p) s -> p k s", p=P))
            wo9_sb = singles.tile([D_STATE + 1, D_MODEL], BF16)
            nc.gpsimd.dma_start(wo9_sb, wo9[:, :])
            wpt_sb = singles.tile([P, KD, D_INNER], BF16)
            wpt_r = wpt[:, :].rearrange("(k p) e -> p k e", p=P)
            for k in range(KD):
                eng = nc.scalar if k % 2 == 0 else nc.gpsimd
                eng.dma_start(wpt_sb[:, k], wpt_r[:, k])

            def a_ln(x_sb, T):
                """layernorm one loaded tile -> xn (token-major, bf16).
                rstd = 1/sqrt(var+eps) via Newton from y0=1 on DVE (LN'd
                variance is ~1) -- keeps the ACT engine on one act table."""
                G = T // P
                xn_sb = xnpool.tile([P, G, D_MODEL], BF16, tag="xn")
                mv = statp.tile([P, G, 2], F32, tag="mv")
                for g in range(G):
                    stats = statp.tile([P, 2, 6], F32, tag="bnst")
                    nc.vector.bn_stats(stats[:, 0, :], x_sb[:, g, 0:512])
                    nc.vector.bn_stats(stats[:, 1, :], x_sb[:, g, 512:1024])
                    nc.vector.bn_aggr(mv[:, g], stats)
                var = mv[:, :, 1]
                ve = statp.tile([P, G], F32, tag="ve")
                nc.vector.tensor_scalar_add(ve, var, EPS)
                y1 = statp.tile([P, G], F32, tag="y1")
                nc.vector.tensor_scalar(
                    out=y1, in0=ve, scalar1=-0.5, scalar2=1.5,
                    op0=mybir.AluOpType.mult, op1=mybir.AluOpType.add,
                )
                sq = statp.tile([P, G], F32, tag="sq")
                nc.vector.tensor_tensor(sq, y1, y1, mybir.AluOpType.mult)
                nc.vector.tensor_tensor(sq, sq, ve, mybir.AluOpType.mult)
                nc.vector.tensor_scalar(
                    out=sq, in0=sq, scalar1=-0.5, scalar2=1.5,
                    op0=mybir.AluOpType.mult, op1=mybir.AluOpType.add,
                )
                rstd = statp.tile([P, G], F32, tag="rstd")
                nc.vector.tensor_tensor(rstd, sq, y1, mybir.AluOpType.mult)
                # second Newton step
                sq2 = statp.tile([P, G], F32, tag="sq2")
                nc.vector.tensor_tensor(sq2, rstd, rstd, mybir.AluOpType.mult)
                nc.vector.tensor_tensor(sq2, sq2, ve, mybir.AluOpType.mult)
                nc.vector.tensor_scalar(
                    out=sq2, in0=sq2, scalar1=-0.5, scalar2=1.5,
                    op0=mybir.AluOpType.mult, op1=mybir.AluOpType.add,
                )
                nc.vector.tensor_tensor(rstd, rstd, sq2, mybir.AluOpType.mult)
                for g in range(G):
                    # normalize on the (otherwise idle) Pool engine: pure
                    # SBUF->SBUF, keeps DVE free for copybacks/residuals
                    nc.gpsimd.tensor_scalar(
                        out=xn_sb[:, g, :],
                        in0=x_sb[:, g, :],
                        scalar1=mv[:, g, 0:1],
                        scalar2=rstd[:, g : g + 1],
                        op0=mybir.AluOpType.subtract,
                        op1=mybir.AluOpType.mult,
                    )
                return xn_sb

            def a_tr(xn_sb, T):
                """transpose to feature-major via regular bf16 matmuls
                (xn.T @ I); gamma/beta fused into the PSUM copyback"""
                G = T // P
                xnT = xtpool.tile([P, KD, G, P], BF16, tag="xnT")
                for k in range(KD):
                    ptr = ps_tr.tile([P, G, P], F32, tag="ptr")
                    for g in range(G):
                        nc.tensor.matmul(
                            ptr[:, g, :],
                            lhsT=xn_sb[:, g, k * P : (k + 1) * P],
                            rhs=ident,
                            start=True,
                            stop=True,
                        )
                    nc.vector.tensor_scalar(
                        out=xnT[:, k],
                        in0=ptr[:],
                        scalar1=gpk_sb[:, k : k + 1],
                        scalar2=bpk_sb[:, k : k + 1],
                        op0=mybir.AluOpType.mult,
                        op1=mybir.AluOpType.add,
                    )
                return xnT

            # software pipeline: x-DMA two tiles ahead, LayerNorm one tile
            # ahead (on DVE during this tile's matmul-1), transposes one tile
            # ahead in the M2->M3 ACT-latency pocket
            xn_cur = a_ln(x_tiles[0], tiles[0][1])
            xnT_cur = a_tr(xn_cur, tiles[0][1])
            xn_next = a_ln(x_tiles[1], tiles[1][1])
            for i, (off, T) in enumerate(tiles):
                x_sb = x_tiles[i]
                xnT = xnT_cur
                G = T // P
                if i + 2 < len(tiles):
                    x_tiles.append(a_dma(*tiles[i + 2]))
                # cs9 row 8 (the b_out ones row) set by the idle Pool engine
                cs9 = statp.tile([D_STATE + 1, TILE_T], BF16, tag="cs9")
                nc.gpsimd.dma_start(cs9[D_STATE : D_STATE + 1, :], ones[:, :])
                # matmul 1: [D_INNER, T] feature-major; SiLU+b_proj fused
                projT = projp.tile([P, ME, TILE_T], BF16, tag="projT")
                for m in range(ME):
                    p1 = ps1.tile([P, TILE_T], F32, tag="p1")
                    for k in range(KD):
                        nc.tensor.matmul(
                            p1[:, :T],
                            lhsT=wpt_sb[:, k, m * P : (m + 1) * P],
                            rhs=xnT[:, k],
                            start=(k == 0),
                            stop=(k == KD - 1),
                        )
                    nc.scalar.activation(
                        out=projT[:, m, :T],
                        in_=p1[:, :T],
                        func=mybir.ActivationFunctionType.Silu,
                        bias=bpm_sb[:, m : m + 1],
                        scale=1.0,
                    )
                # matmul 2: [D_STATE, T]; SiLU+(b_state+init) fused
                p2 = ps2.tile([D_STATE, TILE_T], F32, tag="p2")
                for k2 in range(ME):
                    nc.tensor.matmul(
                        p2[:, :T],
                        lhsT=wst_sb[:, k2, :],
                        rhs=projT[:, k2, :T],
                        start=(k2 == 0),
                        stop=(k2 == ME - 1),
                    )
                # next tile's transposes fill the PE while ACT drains
                # p2 -> cs9; the tile-after's LN is emitted after the m3
                # loop so the residual adds aren't queued behind it on DVE
                if i + 1 < len(tiles):
                    xnT_cur = a_tr(xn_next, tiles[i + 1][1])
                nc.scalar.activation(
                    out=cs9[:D_STATE, :T],
                    in_=p2[:, :T],
                    func=mybir.ActivationFunctionType.Silu,
                    bias=b2_sb,
                    scale=1.0,
                )
                # matmul 3: K=9 (ones row adds b_out), token-major out;
                # residual add fused into the DVE copyback
                out_sb = outp.tile([P, G, D_MODEL], F32, tag="out")
                for g in range(G):
                    for h in range(D_MODEL // 512):
                        p3 = ps3.tile([P, 512], F32, tag="p3")
                        nc.tensor.matmul(
                            p3,
                            lhsT=cs9[:, g * P : (g + 1) * P],
                            rhs=wo9_sb[:, h * 512 : (h + 1) * 512],
                            start=True,
                            stop=True,
                        )
                        nc.vector.tensor_add(
                            out=out_sb[:, g, h * 512 : (h + 1) * 512],
                            in0=p3,
                            in1=x_sb[:, g, h * 512 : (h + 1) * 512],
                        )
                yr = y[off : off + T, :].rearrange("(g p) d -> p g d", p=P)
                for g in range(G):
                    eng = nc.gpsimd if g % 2 == 0 else nc.scalar
                    eng.dma_start(yr[:, g], out_sb[:, g])
                if i + 2 < len(tiles):
                    xn_next = a_ln(x_tiles[i + 2], tiles[i + 2][1])

    _split_multi_waits(nc)
    return nc


_NC_CACHE = None


def _get_nc():
    global _NC_CACHE
    if _NC_CACHE is None:
        _NC_CACHE = build_kernel()
    return _NC_CACHE


def make_in_maps(inputs):
    x = np.ascontiguousarray(inputs["x"], dtype=np.float32).reshape(-1, D_MODEL)
    W_proj = np.asarray(inputs["W_proj"], dtype=np.float32)
    b_proj = np.asarray(inputs["b_proj"], dtype=np.float32)
    W_state = np.asarray(inputs["W_state"], dtype=np.float32)
    b_state = np.asarray(inputs["b_state"], dtype=np.float32)
    W_out = np.asarray(inputs["W_out"], dtype=np.float32)
    b_out = np.asarray(inputs["b_out"], dtype=np.float32)
    initial_state = np.asarray(inputs["initial_state"], dtype=np.float32)
    gamma = np.asarray(inputs["gamma"], dtype=np.float32)
    beta = np.asarray(inputs["beta"], dtype=np.float32)

    bf = ml_dtypes.bfloat16
    shared = {
        "wpt": np.ascontiguousarray(W_proj.T).astype(bf),
        "wst": np.ascontiguousarray(W_state.T).astype(bf),
        "wo9": np.ascontiguousarray(
            np.concatenate([W_out.T, b_out[None, :]], axis=0)
        ).astype(bf),
        "gpk": np.ascontiguousarray(gamma.reshape(KD, P).T),
        "bpk": np.ascontiguousarray(beta.reshape(KD, P).T),
        "bpm": np.ascontiguousarray(b_proj.reshape(ME, P).T),
        "b2": np.ascontiguousarray(
            (b_state + initial_state.reshape(-1)).reshape(D_STATE, 1)
        ),
        "ones": np.ones((1, TILE_T), dtype=bf),
        "ident": np.eye(P, dtype=bf),
    }
    in_maps = []
    for c in range(N_CORES):
        m = {"x": np.ascontiguousarray(x[c * TOK : (c + 1) * TOK])}
        m.update(shared)
        in_maps.append(m)
    return in_maps


def kernel(**inputs) -> np.ndarray:
    nc = _get_nc()
    in_maps = make_in_maps(inputs)
    res = run_bass_kernel_spmd(nc, in_maps, core_ids=list(range(N_CORES)))
    out = np.concatenate([res.results[c]["y"] for c in range(N_CORES)], axis=0)
    return out.reshape(np.asarray(inputs["x"]).shape)


# revision 24
# speedup vs baseline: 1.5407x; 1.5407x over previous
"""Trainium2 Bass kernel for the MemoryEfficientMambaBlock problem.

Data-parallel over 8 NeuronCores: x sharded over tokens, small weights
replicated. Per core, per 256-token tile:
  LayerNorm (bn_stats token-major; rstd via DVE Newton iteration so the
  ACT engine never swaps activation tables) -> transpose to feature-major
  via regular bf16 matmuls against the identity (gamma/beta fused into the
  PSUM copyback, output quantized bf16) -> bf16 matmul x@W_projT with
  SiLU+b_proj fused into the ACT copyback -> bf16 matmul @W_stateT with
  SiLU+(b_state+initial_state) fused -> K=9 bf16 matmul (ones row carries
  b_out) producing token-major output with the residual add fused into the
  DVE copyback.
"""

import sys

if "/opt/trn_rl_repo" not in sys.path:
    sys.path.insert(0, "/opt/trn_rl_repo")

import ml_dtypes
import numpy as np

import concourse.bass as bass
import concourse.mybir as mybir
import concourse.tile as tile
from concourse.bass_utils import run_bass_kernel_spmd

P = 128
D_MODEL = 1024
D_INNER = 2048
D_STATE = 8
EPS = 1e-5
N_CORES = 8
TOK_TOTAL = 2 * 128 * 196  # 50176
TOK = TOK_TOTAL // N_CORES  # 6272
TILE_T = 512

KD = D_MODEL // P  # 8 contraction chunks for matmul 1
ME = D_INNER // P  # 16 output tiles for matmul 1 / contraction chunks for 2

F32 = mybir.dt.float32
BF16 = mybir.dt.bfloat16


def _split_multi_waits(nc):
    """This container's walrus accepts at most ONE semaphore wait per
    instruction. Hoist all but the last wait of each instruction onto
    fresh single-wait NoOps inserted immediately before it on the same
    engine (the sequencer processes instructions in order, so semantics
    are unchanged)."""
    n_split = 0
    for f in nc.m.functions:
        for blk in f.blocks:
            out = []
            changed = False
            for inst in blk.instructions:
                si = inst.sync_info
                waits = list(si.on_wait) if si is not None else []
                if len(waits) > 1:
                    changed = True
                    for j, w in enumerate(waits[:-1]):
                        nop = mybir.InstNoOp(
                            name=f"{inst.name}-wsplit{j}", ins=[], outs=[]
                        )
                        nop.engine = inst.engine
                        nop.sync_info = mybir.SyncInfo(on_wait=[w], on_update=[])
                        out.append(nop)
                        n_split += 1
                    inst.sync_info = mybir.SyncInfo(
                        on_wait=[waits[-1]], on_update=list(si.on_update)
                    )
                out.append(inst)
            if changed:
                blk.instructions = out
    return n_split


def build_kernel():
    nc = bass.Bass()
    x = nc.dram_tensor("x", [TOK, D_MODEL], F32, kind="ExternalInput")
    wpt = nc.dram_tensor("wpt", [D_MODEL, D_INNER], BF16, kind="ExternalInput")
    wst = nc.dram_tensor("wst", [D_INNER, D_STATE], BF16, kind="ExternalInput")
    wo9 = nc.dram_tensor("wo9", [D_STATE + 1, D_MODEL], BF16, kind="ExternalInput")
    gpk = nc.dram_tensor("gpk", [P, KD], F32, kind="ExternalInput")
    bpk = nc.dram_tensor("bpk", [P, KD], F32, kind="ExternalInput")
    bpm = nc.dram_tensor("bpm", [P, ME], F32, kind="ExternalInput")
    b2 = nc.dram_tensor("b2", [D_STATE, 1], F32, kind="ExternalInput")
    ones = nc.dram_tensor("ones", [1, TILE_T], BF16, kind="ExternalInput")
    ident_d = nc.dram_tensor("ident", [P, P], BF16, kind="ExternalInput")
    y = nc.dram_tensor("y", [TOK, D_MODEL], F32, kind="ExternalOutput")

    # bf16 matmuls run at 1 cycle/row at any N, so tiles need not be
    # uniform: small tiles up front shorten the prologue (first matmul
    # starts after ~one LN chain instead of a whole 512-token tile; they
    # also absorb the PE DVFS ramp), the 128-token remainder gets its
    # own tile instead of a recompute-overlap tile
    tiles = [(0, 128), (128, 384)]
    o = 512
    while o + TILE_T <= TOK:
        tiles.append((o, TILE_T))
        o += TILE_T
    if o < TOK:
        tiles.append((o, TOK - o))

    with tile.TileContext(nc) as tc:
        with (
            tc.tile_pool(name="singles", bufs=1) as singles,
            tc.tile_pool(name="xpool", bufs=3) as xpool,
            tc.tile_pool(name="xnpool", bufs=2) as xnpool,
            tc.tile_pool(name="xtpool", bufs=2) as xtpool,
            tc.tile_pool(name="projp", bufs=2) as projp,
            tc.tile_pool(name="outp", bufs=2) as outp,
            tc.tile_pool(name="statp", bufs=6) as statp,
            tc.tile_pool(name="ps_tr", bufs=2, space="PSUM") as ps_tr,
            tc.tile_pool(name="ps1", bufs=2, space="PSUM") as ps1,
            tc.tile_pool(name="ps2", bufs=1, space="PSUM") as ps2,
            tc.tile_pool(name="ps3", bufs=3, space="PSUM") as ps3,
        ):
            # x tiles 0/1 first: LN work can begin before weights finish.
            # One DMA per 128-token group so bn_stats on group g unblocks
            # as soon as its quarter lands (byte-range dep tracking). A
            # single DMA queue moves only ~115 GB/s, so the startup tiles
            # spread their groups across all three queues.
            def a_dma(off, T, spread=False):
                G = T // P
                x_sb = xpool.tile([P, G, D_MODEL], F32, tag="x")
                xr = x[off : off + T, :].rearrange("(g p) d -> p g d", p=P)
                engs = [nc.sync, nc.scalar] if spread else [nc.sync]
                for g in range(G):
                    engs[g % len(engs)].dma_start(x_sb[:, g], xr[:, g])
                return x_sb

            x_tiles = [a_dma(*tiles[0], spread=True), a_dma(*tiles[1])]

            # weights spread across the scalar/gpsimd DMA queues so they
            # load in parallel with the first x tiles on sync; small
            # tensors first -- the LN->transpose->copyback chain needs
            # ident/gpk/bpk before any wpt chunk is touched
            ident = singles.tile([P, P], BF16)
            nc.gpsimd.dma_start(ident, ident_d[:, :])
            gpk_sb = singles.tile([P, KD], F32)
            nc.scalar.dma_start(gpk_sb, gpk[:, :])
            bpk_sb = singles.tile([P, KD], F32)
            nc.scalar.dma_start(bpk_sb, bpk[:, :])
            bpm_sb = singles.tile([P, ME], F32)
            nc.gpsimd.dma_start(bpm_sb, bpm[:, :])
            b2_sb = singles.tile([D_STATE, 1], F32)
            nc.scalar.dma_start(b2_sb, b2[:, :])
            wst_sb = singles.tile([P, ME, D_STATE], BF16)
            nc.gpsimd.dma_start(wst_sb, wst[:, :].rearrange("(k p) s -> p k s", p=P))
            wo9_sb = singles.tile([D_STATE + 1, D_MODEL], BF16)
            nc.gpsimd.dma_start(wo9_sb, wo9[:, :])
            wpt_sb = singles.tile([P, KD, D_INNER], BF16)
            wpt_r = wpt[:, :].rearrange("(k p) e -> p k e", p=P)
            for k in range(KD):
                eng = nc.scalar if k % 2 == 0 else nc.gpsimd
                eng.dma_start(wpt_sb[:, k], wpt_r[:, k])

            def a_ln(x_sb, T):
                """layernorm one loaded tile -> xn (token-major, bf16).
                rstd = 1/sqrt(var+eps) via Newton from y0=1 on DVE (LN'd
                variance is ~1) -- keeps the ACT engine on one act table."""
                G = T // P
                xn_sb = xnpool.tile([P, G, D_MODEL], BF16, tag="xn")
                mv = statp.tile([P, G, 2], F32, tag="mv")
                for g in range(G):
                    stats = statp.tile([P, 2, 6], F32, tag="bnst")
                    nc.vector.bn_stats(stats[:, 0, :], x_sb[:, g, 0:512])
                    nc.vector.bn_stats(stats[:, 1, :], x_sb[:, g, 512:1024])
                    nc.vector.bn_aggr(mv[:, g], stats)
                var = mv[:, :, 1]
                ve = statp.tile([P, G], F32, tag="ve")
                nc.vector.tensor_scalar_add(ve, var, EPS)
                y1 = statp.tile([P, G], F32, tag="y1")
                nc.vector.tensor_scalar(
                    out=y1, in0=ve, scalar1=-0.5, scalar2=1.5,
                    op0=mybir.AluOpType.mult, op1=mybir.AluOpType.add,
                )
                sq = statp.tile([P, G], F32, tag="sq")
                nc.vector.tensor_tensor(sq, y1, y1, mybir.AluOpType.mult)
                nc.vector.tensor_tensor(sq, sq, ve, mybir.AluOpType.mult)
                nc.vector.tensor_scalar(
                    out=sq, in0=sq, scalar1=-0.5, scalar2=1.5,
                    op0=mybir.AluOpType.mult, op1=mybir.AluOpType.add,
                )
                rstd = statp.tile([P, G], F32, tag="rstd")
                nc.vector.tensor_tensor(rstd, sq, y1, mybir.AluOpType.mult)
                # second Newton step
                sq2 = statp.tile([P, G], F32, tag="sq2")
                nc.vector.tensor_tensor(sq2, rstd, rstd, mybir.AluOpType.mult)
                nc.vector.tensor_tensor(sq2, sq2, ve, mybir.AluOpType.mult)
                nc.vector.tensor_scalar(
                    out=sq2, in0=sq2, scalar1=-0.5, scalar2=1.5,
                    op0=mybir.AluOpType.mult, op1=mybir.AluOpType.add,
                )
                nc.vector.tensor_tensor(rstd, rstd, sq2, mybir.AluOpType.mult)
                for g in range(G):
                    nc.vector.tensor_scalar(
                        out=xn_sb[:, g, :],
                        in0=x_sb[:, g, :],
                        scalar1=mv[:, g, 0:1],
                        scalar2=rstd[:, g : g + 1],
                        op0=mybir.AluOpType.subtract,
                        op1=mybir.AluOpType.mult,
                    )
                return xn_sb

            def a_tr(xn_sb, T):
                """PE transpose to feature-major (bf16 in AND out of PSUM,
                so the gamma/beta copyback runs at the 16-bit DVE rate)"""
                G = T // P
                xnT = xtpool.tile([P, KD, G, P], BF16, tag="xnT")
                for k in range(KD):
                    ptr = ps_tr.tile([P, G, P], BF16, tag="ptr")
                    for g in range(G):
                        nc.tensor.transpose(
                            ptr[:, g, :],
                            xn_sb[:, g, k * P : (k + 1) * P],
                            ident,
                        )
                    nc.vector.tensor_scalar(
                        out=xnT[:, k],
                        in0=ptr[:],
                        scalar1=gpk_sb[:, k : k + 1],
                        scalar2=bpk_sb[:, k : k + 1],
                        op0=mybir.AluOpType.mult,
                        op1=mybir.AluOpType.add,
                    )
                return xnT

            # software pipeline: x-DMA two tiles ahead, LayerNorm one tile
            # ahead (on DVE during this tile's matmul-1), transposes one tile
            # ahead in the M2->M3 ACT-latency pocket
            xn_cur = a_ln(x_tiles[0], tiles[0][1])
            xnT_cur = a_tr(xn_cur, tiles[0][1])
            xn_next = a_ln(x_tiles[1], tiles[1][1])
            for i, (off, T) in enumerate(tiles):
                x_sb = x_tiles[i]
                xnT = xnT_cur
                G = T // P
                if i + 2 < len(tiles):
                    x_tiles.append(a_dma(*tiles[i + 2]))
                # cs9 row 8 (the b_out ones row) set by the idle Pool engine
                cs9 = statp.tile([D_STATE + 1, TILE_T], BF16, tag="cs9")
                nc.gpsimd.dma_start(cs9[D_STATE : D_STATE + 1, :], ones[:, :])
                # matmul 1: [D_INNER, T] feature-major; SiLU+b_proj fused
                projT = projp.tile([P, ME, TILE_T], BF16, tag="projT")
                for m in range(ME):
                    p1 = ps1.tile([P, TILE_T], F32, tag="p1")
                    for k in range(KD):
                        nc.tensor.matmul(
                            p1[:, :T],
                            lhsT=wpt_sb[:, k, m * P : (m + 1) * P],
                            rhs=xnT[:, k],
                            start=(k == 0),
                            stop=(k == KD - 1),
                        )
                    nc.scalar.activation(
                        out=projT[:, m, :T],
                        in_=p1[:, :T],
                        func=mybir.ActivationFunctionType.Silu,
                        bias=bpm_sb[:, m : m + 1],
                        scale=1.0,
                    )
                # matmul 2: [D_STATE, T]; SiLU+(b_state+init) fused
                p2 = ps2.tile([D_STATE, TILE_T], F32, tag="p2")
                for k2 in range(ME):
                    nc.tensor.matmul(
                        p2[:, :T],
                        lhsT=wst_sb[:, k2, :],
                        rhs=projT[:, k2, :T],
                        start=(k2 == 0),
                        stop=(k2 == ME - 1),
                    )
                # next tile's transposes fill the PE while ACT drains
                # p2 -> cs9; the tile-after's LN is emitted after the m3
                # loop so the residual adds aren't queued behind it on DVE
                if i + 1 < len(tiles):
                    xnT_cur = a_tr(xn_next, tiles[i + 1][1])
                nc.scalar.activation(
                    out=cs9[:D_STATE, :T],
                    in_=p2[:, :T],
                    func=mybir.ActivationFunctionType.Silu,
                    bias=b2_sb,
                    scale=1.0,
                )
                # matmul 3: K=9 (ones row adds b_out), token-major out;
                # residual add fused into the DVE copyback
                out_sb = outp.tile([P, G, D_MODEL], F32, tag="out")
                for g in range(G):
                    for h in range(D_MODEL // 512):
                        p3 = ps3.tile([P, 512], F32, tag="p3")
                        nc.tensor.matmul(
                            p3,
                            lhsT=cs9[:, g * P : (g + 1) * P],
                            rhs=wo9_sb[:, h * 512 : (h + 1) * 512],
                            start=True,
                            stop=True,
                        )
                        nc.vector.tensor_add(
                            out=out_sb[:, g, h * 512 : (h + 1) * 512],
                            in0=p3,
                            in1=x_sb[:, g, h * 512 : (h + 1) * 512],
                        )
                yr = y[off : off + T, :].rearrange("(g p) d -> p g d", p=P)
                for g in range(G):
                    eng = nc.gpsimd if g % 2 == 0 else nc.scalar
                    eng.dma_start(yr[:, g], out_sb[:, g])
                if i + 2 < len(tiles):
                    xn_next = a_ln(x_tiles[i + 2], tiles[i + 2][1])

    _split_multi_waits(nc)
    return nc


_NC_CACHE = None


def _get_nc():
    global _NC_CACHE
    if _NC_CACHE is None:
        _NC_CACHE = build_kernel()
    return _NC_CACHE


def make_in_maps(inputs):
    x = np.ascontiguousarray(inputs["x"], dtype=np.float32).reshape(-1, D_MODEL)
    W_proj = np.asarray(inputs["W_proj"], dtype=np.float32)
    b_proj = np.asarray(inputs["b_proj"], dtype=np.float32)
    W_state = np.asarray(inputs["W_state"], dtype=np.float32)
    b_state = np.asarray(inputs["b_state"], dtype=np.float32)
    W_out = np.asarray(inputs["W_out"], dtype=np.float32)
    b_out = np.asarray(inputs["b_out"], dtype=np.float32)
    initial_state = np.asarray(inputs["initial_state"], dtype=np.float32)
    gamma = np.asarray(inputs["gamma"], dtype=np.float32)
    beta = np.asarray(inputs["beta"], dtype=np.float32)

    bf = ml_dtypes.bfloat16
    shared = {
        "wpt": np.ascontiguousarray(W_proj.T).astype(bf),
        "wst": np.ascontiguousarray(W_state.T).astype(bf),
        "wo9": np.ascontiguousarray(
            np.concatenate([W_out.T, b_out[None, :]], axis=0)
        ).astype(bf),
        "gpk": np.ascontiguousarray(gamma.reshape(KD, P).T),
        "bpk": np.ascontiguousarray(beta.reshape(KD, P).T),
        "bpm": np.ascontiguousarray(b_proj.reshape(ME, P).T),
        "b2": np.ascontiguousarray(
            (b_state + initial_state.reshape(-1)).reshape(D_STATE, 1)
        ),
        "ones": np.ones((1, TILE_T), dtype=bf),
        "ident": np.eye(P, dtype=bf),
    }
    in_maps = []
    for c in range(N_CORES):
        m = {"x": np.ascontiguousarray(x[c * TOK : (c + 1) * TOK])}
        m.update(shared)
        in_maps.append(m)
    return in_maps


def kernel(**inputs) -> np.ndarray:
    nc = _get_nc()
    in_maps = make_in_maps(inputs)
    res = run_bass_kernel_spmd(nc, in_maps, core_ids=list(range(N_CORES)))
    out = np.concatenate([res.results[c]["y"] for c in range(N_CORES)], axis=0)
    return out.reshape(np.asarray(inputs["x"]).shape)
